# revision 2
# baseline (speedup 1.0000x reference)
"""Trainium2 Bass kernel: 11x11 valid cross-correlation, 6144x6144 fp32,
SPMD 8 cores, S=12 column-phase decomposition (same math as v4).

Differences vs v4:
  - Variable-size tile groups ([2,2,4,8...,5]) with per-group input DMA
    chunks so the first matmul starts after ~1.5us instead of ~6us.
  - Stationary-major matmul ordering inside each group (all tiles for one
    stationary before switching) so the PE weight reload happens once per
    G matmuls instead of every matmul.
  - Boundary block comes from the previous group's SBUF tile (no re-DMA).
  - Last group's output DMAed per-tile to shrink the tail.

Math (unchanged): with 12 column phases and 10-row blocks, K = 120,
tile j (j=1..77) covers output rows 10j-10..10j-1 via 4 matmuls:
  set1 s=0,1 from block j, set2 s=0,1 from block j-1, all N=512.
"""

import time

import numpy as np
import ml_dtypes

try:
    from concourse import bacc, mybir
except ImportError:
    import sys
    sys.path.insert(0, "/opt/trn_rl_repo")
    from concourse import bacc, mybir
import concourse.tile as tile
from concourse.bass_utils import run_bass_kernel_spmd

KH = KW = 11
H = W = 6144
OH = OW = H - (KH - 1)          # 6134

N_CORES = 8
S = 12                          # column phases
RW = 10                         # rows per block
K = RW * S                      # 120 contraction partitions
M1 = RW * S                     # 120 used output partitions
MP = 128                        # padded stationary columns (FWL)

CORE_OUT = 768                  # output rows per core
NTIL = 77                       # output tiles j=1..77
NBLK = 78                       # input 10-row blocks per core (0..77)
ROWS_IN = NBLK * RW             # 780 input rows per core (zero-padded)
NMB = 513                       # m-positions per block slab (512 + s-shift)
NOB = 512                       # output m-blocks per tile row

# tile groups: (j0, G) — tiles j0..j0+G-1 processed stationary-major
TGROUPS = [(1, 2), (3, 2), (5, 4), (9, 8), (17, 8), (25, 8), (33, 8),
           (41, 8), (49, 8), (57, 8), (65, 8), (73, 5)]
assert sum(g for _, g in TGROUPS) == NTIL

_prog_cache: dict = {}


def _build_program(reps: int = 1, timing: bool = False):
    key = (reps, timing)
    if key in _prog_cache:
        return _prog_cache[key]

    bf16 = mybir.dt.bfloat16
    f32 = mybir.dt.float32
    nc = bacc.Bacc("TRN2", target_bir_lowering=False, debug=False,
                   num_devices=N_CORES)

    if timing:
        xp = nc.dram_tensor("xp", [K, NBLK * NMB], bf16).ap()
        outp = nc.dram_tensor("outp", [NTIL * M1, NOB], bf16).ap()
        tout = nc.dram_tensor("tout", [NTIL, 64], bf16,
                              kind="ExternalOutput").ap()
    else:
        xp = nc.dram_tensor("xp", [K, NBLK * NMB], bf16,
                            kind="ExternalInput").ap()
        outp = nc.dram_tensor("outp", [NTIL * M1, NOB], bf16,
                              kind="ExternalOutput").ap()
    tw = nc.dram_tensor("tw", [K, 4 * MP], bf16, kind="ExternalInput").ap()

    # input DMA chunks: group 0 -> blocks [0, j0+G-1]; group k -> [j0, j0+G-1]
    chunks = []
    for gi, (j0, G) in enumerate(TGROUPS):
        lo = 0 if gi == 0 else j0
        chunks.append((lo, j0 + G - 1))
    assert chunks[-1][1] == NBLK - 1

    with tile.TileContext(nc) as tc:
        with (
            tc.tile_pool(name="twp", bufs=1) as twp,
            tc.tile_pool(name="xpool", bufs=5) as xpool,
            tc.tile_pool(name="pspool", bufs=8, space="PSUM") as pspool,
            tc.tile_pool(name="opool", bufs=3) as opool,
        ):
            twt = twp.tile([K, 4 * MP], bf16)
            nc.sync.dma_start(twt[:], tw[:])

            for _ in range(reps):
                xts = []                      # per-group (tile, lo, hi)
                for gi, (j0, G) in enumerate(TGROUPS):
                    lo, hi = chunks[gi]
                    nb = hi - lo + 1
                    xt = xpool.tile([K, 8 * NMB], bf16, name="xg")
                    nc.sync.dma_start(
                        xt[:, :nb * NMB],
                        xp[:, lo * NMB:(hi + 1) * NMB])
                    xts.append((xt, lo, hi))

                def mov(blk, shift):
                    """moving AP for block blk, column shift 0/1"""
                    for xt, lo, hi in xts:
                        if lo <= blk <= hi:
                            off = (blk - lo) * NMB + shift
                            return xt[:, off:off + NOB]
                    raise AssertionError(blk)

                for gi, (j0, G) in enumerate(TGROUPS):
                    pss = []
                    for t in range(G):
                        psb = pspool.tile([MP, NOB], f32, tag="psb",
                                          name="psb")
                        pss.append(psb)
                    # stationary-major: (tw offset, use prev block, shift)
                    for si, (twoff, useprev, shift) in enumerate([
                            (2 * MP, True, 0), (3 * MP, True, 1),
                            (0, False, 0), (MP, False, 1)]):
                        for t in range(G):
                            j = j0 + t
                            blk = j - 1 if useprev else j
                            nc.tensor.matmul(
                                pss[t][:], twt[:, twoff:twoff + MP],
                                mov(blk, shift),
                                start=(si == 0), stop=(si == 3),
                                skip_group_check=True)
                    ot = opool.tile([K, 8 * NOB], bf16, name="ot")
                    for t in range(G):
                        if t % 2 == 0:
                            nc.vector.tensor_copy(
                                ot[:, t * NOB:(t + 1) * NOB], pss[t][:K, :])
                        else:
                            nc.scalar.copy(
                                ot[:, t * NOB:(t + 1) * NOB], pss[t][:K, :])
                    last = gi == len(TGROUPS) - 1
                    if not last:
                        nc.scalar.dma_start(
                            outp[(j0 - 1) * M1:(j0 - 1 + G) * M1,
                                 :].rearrange("(b k) m -> k b m", k=M1),
                            ot[:, :G * NOB].rearrange(
                                "k (b m) -> k b m", b=G))
                    else:
                        for t in range(G):
                            j = j0 + t
                            nc.scalar.dma_start(
                                outp[(j - 1) * M1:j * M1, :],
                                ot[:, t * NOB:(t + 1) * NOB])

            if timing:
                nc.sync.dma_start(tout[:, :], outp[0:NTIL * M1:M1, 0:64])

    nc.compile()
    _prog_cache[key] = nc
    return nc


def _build_tw(weight: np.ndarray) -> np.ndarray:
    """[K, 4*MP] bf16 stationary: [set1 s0 | set1 s1 | set2 s0 | set2 s1]."""
    w_ = np.asarray(weight, np.float32)
    tb1 = np.zeros((2, K, MP), np.float32)
    tb2 = np.zeros((2, K, MP), np.float32)
    for s in range(2):
        for w in range(RW):
            for p in range(S):
                for q in range(S):
                    v = p - q + S * s
                    if not (0 <= v <= KW - 1):
                        continue
                    for i in range(RW):
                        u = w + 10 - i
                        if 0 <= u <= KH - 1:
                            tb1[s, w * S + p, i * S + q] = w_[u, v]
                        u = w - i
                        if 0 <= u <= KH - 1:
                            tb2[s, w * S + p, i * S + q] = w_[u, v]
    return np.ascontiguousarray(np.concatenate(
        [tb1[0], tb1[1], tb2[0], tb2[1]], axis=1)).astype(ml_dtypes.bfloat16)


def _shard_inputs(X: np.ndarray, weight: np.ndarray):
    Xf = np.asarray(X, np.float32)
    twc = _build_tw(weight)
    in_maps = []
    for kcore in range(N_CORES):
        r0 = CORE_OUT * kcore
        xs = np.zeros((ROWS_IN, NMB * S), np.float32)   # 780 x 6156
        n = min(ROWS_IN, H - r0)
        xs[:n, :W] = Xf[r0:r0 + n]
        arr = xs.reshape(NBLK, RW, NMB, S)              # [b, r, m, p]
        xpk = np.ascontiguousarray(
            arr.transpose(1, 3, 0, 2)).reshape(K, NBLK * NMB).astype(
            ml_dtypes.bfloat16)
        in_maps.append({"xp": xpk, "tw": twc})
    return in_maps


def _assemble_output(results, bias_val: float) -> np.ndarray:
    out = np.empty((OH, OW), np.float32)
    for kcore in range(N_CORES):
        op = np.asarray(results[kcore]["outp"], np.float32).reshape(
            NTIL, RW, S, NOB)                           # [j', i, q, m]
        rows = np.ascontiguousarray(op.transpose(0, 1, 3, 2)).reshape(
            NTIL * RW, S * NOB)                         # row 10j'+i, col 12m+q
        r0 = CORE_OUT * kcore
        take = min(CORE_OUT, OH - r0)
        out[r0:r0 + take] = rows[:take, :OW]
    if bias_val != 0.0:
        out += bias_val
    return out


def kernel(X: np.ndarray, weight: np.ndarray, bias: np.ndarray) -> np.ndarray:
    nc = _build_program(reps=1)
    in_maps = _shard_inputs(X, weight)
    last_err = None
    for attempt in range(4):
        try:
            res = run_bass_kernel_spmd(nc, in_maps, list(range(N_CORES)))
            break
        except Exception as e:  # transient device wedge: wait and retry
            last_err = e
            time.sleep(90)
    else:
        raise last_err
    return _assemble_output(res.results, float(np.asarray(bias).reshape(-1)[0]))
